# revision 5
# baseline (speedup 1.0000x reference)
"""Correlation (FlowNet-style, max_displacement=4) on 8 TRN2 NeuronCores.

Full inputs x1, x2: [B=8, C=64, H=192, W=192] fp32. Output: [8, 81, 192, 192] fp32.
out[b, di*9+dj, h, w] = mean_c x1[b,c,h,w] * x2pad[b,c,h+di,w+dj]   (di,dj in [0,9))

Strategy: batch-parallel (1 batch per core). Per core the correlation is computed
as a banded Gram matrix on the TensorEngine: for each 8x16 (h,w) output tile, one
bf16 matmul with lhsT = x1 tile [K=64 channels, M=128 pixels] and rhs = padded x2
window [64, 16*24=384 pixels] produces all 81 displacement dot products of every
tile pixel inside a skewed band of the 128x384 PSUM result. PSUM is copied
(fp32->bf16) to SBUF by DVE/ACT and DMA'd to DRAM; the band is deskewed on the
host with a zero-copy strided view. x1 is pre-scaled by 1/64 on the host (exact,
power of two) so the matmul output is directly the channel mean.

The h axis is split into two halves living on partitions 0-63 / 64-127, which
keeps DMA at full 128-partition width and lets the paired matmuls run
concurrently on disjoint PE row-groups (K=64 each).
"""

import sys
import types

import numpy as np
import ml_dtypes

import concourse.bacc as bacc
from concourse import mybir
from concourse.tile import TileContext
from concourse.bass_utils import run_bass_kernel_spmd

B, C, H, W = 8, 64, 192, 192
MAXD = 4
D = 2 * MAXD + 1  # 9
HP, WP = H + 2 * MAXD, W + 2 * MAXD  # 200, 200

TH, TW = 8, 16            # output tile (h, w) -> M = 128
NH, NW = TH + 2 * MAXD, TW + 2 * MAXD  # x2 window 16 x 24 -> N = 384
N_STRIPS = H // TH        # 24 h-strips
N_WT = W // TW            # 12 w-tiles
HHALF = H // 2            # 96 rows per partition-half
SLAB = HHALF + 2 * MAXD   # 104 padded x2 rows per half

BF16 = ml_dtypes.bfloat16


def _install_axon_trace_shim():
    """The image's antenv package lacks axon_hooks; run_bass_kernel_spmd
    crashes on import when trace=True. Provide the hook from the boot module
    so tracing works instead of raising."""
    if "antenv.axon_hooks" in sys.modules:
        return
    try:
        import trn_agent_boot.trn_boot as tb

        hook = tb._ntff_profile_via_ctypes("/opt/axon/libaxon_pjrt.so")
    except Exception:
        hook = None
    mod = types.ModuleType("antenv.axon_hooks")
    mod.get_axon_ntff_profile_hook = lambda: hook
    mod.set_axon_ntff_profile_hook = lambda h: None
    sys.modules["antenv.axon_hooks"] = mod


def build_nc():
    nc = bacc.Bacc("TRN2", target_bir_lowering=False, debug=False)
    # x1 arrives pre-tiled: [128, strip, wtile, 128 pixels] — walrus requires
    # the matmul weights AP to have a single free dimension.
    x1s = nc.dram_tensor("x1s", [128, N_STRIPS // 2, N_WT, TH * TW], mybir.dt.bfloat16, kind="ExternalInput")
    x2s = nc.dram_tensor("x2s", [128, SLAB, WP], mybir.dt.bfloat16, kind="ExternalInput")
    y = nc.dram_tensor("y", [N_STRIPS, 128, N_WT, NH * NW], mybir.dt.bfloat16, kind="ExternalOutput")

    with TileContext(nc) as tc:
        with (
            tc.tile_pool(name="imgs", bufs=1) as imgs,
            tc.tile_pool(name="outs", bufs=3) as outs,
            tc.tile_pool(name="psum", bufs=8, space="PSUM") as psum,
        ):
            x1_sb = imgs.tile([128, N_STRIPS // 2, N_WT, TH * TW], mybir.dt.bfloat16)
            x2_sb = imgs.tile([128, SLAB, WP], mybir.dt.bfloat16)
            nc.sync.dma_start(out=x1_sb[:], in_=x1s[:])
            nc.sync.dma_start(out=x2_sb[:], in_=x2s[:])

            # Strips 0..11 use partition half 0 (h in [0,96)), 12..23 half 1.
            # Emit the two halves' matmuls adjacently: disjoint PE row groups
            # (K=64 at base partitions 0 / 64) execute concurrently.
            for sp in range(N_STRIPS // 2):
                hl = sp * TH  # local h offset within the half
                ybufs = []
                for half in range(2):
                    p0 = 64 * half
                    ybuf = outs.tile([128, N_WT, NH * NW], mybir.dt.bfloat16,
                                     tag=f"ybuf{half}")
                    ybufs.append(ybuf)
                    for t in range(N_WT):
                        w0 = t * TW
                        pt = psum.tile([128, NH * NW], mybir.dt.float32)
                        nc.tensor.matmul(
                            pt[:],
                            lhsT=x1_sb[p0:p0 + 64, sp, t, :],
                            rhs=x2_sb[p0:p0 + 64, hl:hl + NH, w0:w0 + NW],
                            start=True, stop=True,
                        )
                        # Alternate PSUM eviction between DVE and ACT.
                        if (t + half) % 2 == 0:
                            nc.vector.tensor_copy(ybuf[:, t, :], pt[:])
                        else:
                            nc.scalar.copy(ybuf[:, t, :], pt[:])
                for half in range(2):
                    nc.sync.dma_start(out=y[sp + 12 * half], in_=ybufs[half][:])

    nc.compile()
    return nc


_NC_CACHE = None


def _get_nc():
    global _NC_CACHE
    if _NC_CACHE is None:
        _NC_CACHE = build_nc()
    return _NC_CACHE


def _prep_inputs(x1, x2):
    """Host-side shard prep: scale, pad, split h into partition halves, bf16."""
    in_maps = []
    x1 = np.asarray(x1, dtype=np.float32)
    x2 = np.asarray(x2, dtype=np.float32)
    x1h = (x1 * (1.0 / C)).astype(BF16)
    x2h = x2.astype(BF16)
    for b in range(B):
        # x1: [64, 192, 192] -> pre-tiled [128 = half*64+c, sp, t, dh*16+dw]
        a = x1h[b].reshape(C, 2, N_STRIPS // 2, TH, N_WT, TW)
        a = a.transpose(1, 0, 2, 4, 3, 5).reshape(128, N_STRIPS // 2, N_WT, TH * TW)
        # x2: pad to [64, 200, 200], two overlapping 104-row slabs
        p = np.zeros((C, HP, WP), dtype=BF16)
        p[:, MAXD:MAXD + H, MAXD:MAXD + W] = x2h[b]
        s = np.stack([p[:, 0:SLAB, :], p[:, HHALF:HHALF + SLAB, :]], axis=0)
        s = s.reshape(2 * C, SLAB, WP)
        in_maps.append({"x1s": np.ascontiguousarray(a), "x2s": np.ascontiguousarray(s)})
    return in_maps


def _deskew(yb):
    """yb: [24, 128, 12, 384] fp32 (one batch) -> [81, 192, 192] fp32."""
    e = yb.strides[-1]  # element stride in bytes
    s_s, s_p, s_t, _ = yb.strides
    s_dh = 16 * s_p     # partition = dh*16 + dw
    s_dw = s_p
    s_dh2 = NW * e      # column = dh2*24 + dw2
    v = np.lib.stride_tricks.as_strided(
        yb,
        shape=(D, D, N_STRIPS, TH, N_WT, TW),
        strides=(s_dh2, e, s_s, s_dh + s_dh2, s_t, s_dw + e),
    )
    return np.ascontiguousarray(v).reshape(D * D, H, W)


def kernel(x1, x2):
    _install_axon_trace_shim()
    nc = _get_nc()
    in_maps = _prep_inputs(x1, x2)
    res = run_bass_kernel_spmd(nc, in_maps, core_ids=list(range(B)))
    kernel.last_results = res
    out = np.empty((B, D * D, H, W), dtype=np.float32)
    for b in range(B):
        yb = np.asarray(res.results[b]["y"]).astype(np.float32)
        out[b] = _deskew(yb)
    return out
